# revision 33
# baseline (speedup 1.0000x reference)
"""ViT transformer block (B=64, N=197, D=768, H=12, MLP 3072) on 8 trn2 cores.

Data-parallel over batch (8 images per core). Per core:
  - LayerNorm affine terms folded into the following matmul weights (host).
  - Decoupled rel-pos bias folded into the QK matmul via 30 extra contraction
    dims (one-hot row/col encodings x bias-table slices): scores leave the PE
    with the bias already added.
  - Scores computed transposed (sT[kt, qt]); softmax denominators fall out of
    the AV matmul via a block of 64 ones columns appended to V (AV output rows
    64:128 = broadcast denominators); normalize is one DVE divide per head.
  - q scale folded into Wq; v_bias folded into proj bias (host).
  - bf16 operands into the PE, fp32 accumulation in PSUM.
"""

import numpy as np
import ml_dtypes

import concourse.bass as bass
import concourse.mybir as mybir
import concourse.tile as tile
from concourse import bacc
from concourse.bass_utils import run_bass_kernel_spmd
from concourse.masks import make_identity

F32 = mybir.dt.float32
BF16 = mybir.dt.bfloat16
FP8 = mybir.dt.float8e4
NPBF16 = ml_dtypes.bfloat16
NPFP8 = ml_dtypes.float8_e4m3

DIM = 768
HEADS = 12
HD = 64
W0 = 14
W1 = 14
NT = W0 * W1
N = NT + 1  # 197
HID = 4 * DIM  # 3072
B = 64
SCALE = HD ** -0.5
EPS = 1e-6

NCORES = 8
NB = B // NCORES            # 8 images per core
NTOK = NB * N               # 1576
NTILES = 13                 # token tiles of 128
NTOKP = NTILES * 128        # 1664
KEXT = 30                   # extra contraction dims carrying the rel-pos bias
NSL = [512, 512, 512, 128]  # token-column slices of NTOKP
NSL2 = [256] * 6 + [128]    # MLP token-column slices

_nc_cache = {}


def _host_prep(inp):
    """Fold norms/scale/biases; build the rel-pos extension tables."""
    f32 = np.float32
    qkv_w = np.asarray(inp["qkv_w"], f32)
    n1w = np.asarray(inp["norm1_w"], f32)
    n1b = np.asarray(inp["norm1_b"], f32)
    q_bias = np.asarray(inp["q_bias"], f32)
    v_bias = np.asarray(inp["v_bias"], f32)
    proj_w = np.asarray(inp["proj_w"], f32)
    proj_b = np.asarray(inp["proj_b"], f32)
    n2w = np.asarray(inp["norm2_w"], f32)
    n2b = np.asarray(inp["norm2_b"], f32)
    fc1_w = np.asarray(inp["fc1_w"], f32)
    fc1_b = np.asarray(inp["fc1_b"], f32)
    fc2_w = np.asarray(inp["fc2_w"], f32)
    fc2_b = np.asarray(inp["fc2_b"], f32)
    rpb_h = np.asarray(inp["rpb_high"], f32)   # [30, 12]
    rpb_w = np.asarray(inp["rpb_width"], f32)  # [30, 12]

    # qkv with norm1 affine folded; q part pre-scaled
    w_qkv = qkv_w * n1w[None, :]                      # [2304, 768]
    b_qkv = qkv_w @ n1b
    b_qkv[:DIM] += q_bias
    b_qkv[2 * DIM:] += v_bias
    w_qkv[:DIM] *= SCALE
    b_qkv[:DIM] *= SCALE
    wqkv_h = np.ascontiguousarray(
        w_qkv.T.reshape(6, 128, 3 * DIM).transpose(1, 0, 2)).astype(NPBF16)
    qkb_h = np.ascontiguousarray(
        b_qkv[:2 * DIM].reshape(12, 128).T).astype(f32)   # [128, 12]

    # proj; v_bias folded into bias
    pb = proj_b + proj_w @ v_bias                      # [768]
    wproj_h = np.ascontiguousarray(
        proj_w.T.reshape(6, 128, DIM).transpose(1, 0, 2)).astype(NPBF16)

    # fc1 with norm2 folded
    w1 = fc1_w * n2w[None, :]
    b1 = fc1_b + fc1_w @ n2b                           # [3072]
    w1_scale = float(2.0 ** np.floor(np.log2(120.0 / np.abs(w1).max())))
    w1_h = np.ascontiguousarray(
        (w1.T * w1_scale).reshape(6, 128, HID).transpose(1, 0, 2)).astype(NPFP8)
    b1_h = np.ascontiguousarray(b1.reshape(24, 128).T).astype(f32)  # [128, 24]

    w2_scale = float(2.0 ** np.floor(np.log2(120.0 / np.abs(fc2_w).max())))
    w2_h = np.ascontiguousarray(
        (fc2_w.T * w2_scale).reshape(24, 128, DIM).transpose(1, 0, 2)).astype(NPFP8)
    f2b = fc2_b.astype(f32)
    has_f2b = bool(np.any(f2b != 0.0))

    # --- rel-pos bias factorization ---------------------------------------
    # bias[h,q,k] = rpb_h[high_idx[q,k],h] + rpb_w[width_idx[q,k],h];
    # interior: high_idx = krow-qrow+13. CLS handled by dims 28/29.
    qext = np.zeros((KEXT, N), f32)
    for t in range(N):
        if t == 0:
            qext[28, t] = 1.0
        else:
            p = t - 1
            qext[p // W1, t] = 1.0
            qext[14 + p % W1, t] = 1.0
            qext[29, t] = 1.0
    kext = np.zeros((HEADS, KEXT, N), f32)
    for t in range(N):
        if t == 0:
            kext[:, 28, t] = rpb_h[2 * W0 + 1] + rpb_w[2 * W1 + 1]   # corner
            kext[:, 29, t] = rpb_h[2 * W0] + rpb_w[2 * W1]
        else:
            p = t - 1
            kr, kc = p // W1, p % W1
            for rq in range(W0):
                kext[:, rq, t] = rpb_h[kr - rq + W0 - 1]
            for cq in range(W1):
                kext[:, 14 + cq, t] = rpb_w[kc - cq + W1 - 1]
            kext[:, 28, t] = rpb_h[2 * W0 - 1] + rpb_w[2 * W1 - 1]

    return {
        "wqkv": wqkv_h, "qkb": qkb_h, "wproj": wproj_h,
        "pb_row": pb.astype(f32),
        "w1": w1_h, "b1c": b1_h, "w2": w2_h,
        "f2b": np.ascontiguousarray(
            np.broadcast_to(f2b, (128, DIM))),
        "has_f2b": has_f2b, "w1_scale": w1_scale,
        "w2_scale": w2_scale,
        "qext": np.ascontiguousarray(
            np.repeat(qext[:, None, :], HEADS, axis=1)).astype(NPBF16),
        "kext": np.ascontiguousarray(kext.transpose(1, 0, 2)).astype(NPBF16),
    }


def _ln_apply(nc, pool, x_ap, out_ap, eps_col):
    """LayerNorm (no affine) of x_ap [128, 768] -> out_ap bf16.

    rstd = exp(-0.5*ln(var+eps)) on Act (keeps the exp/ln table loaded;
    no Sqrt table swaps), apply = Act identity with per-partition
    scale/bias."""
    Act = mybir.ActivationFunctionType
    stats = pool.tile([128, 3, 6], F32, tag="ln_stats")
    for sg in range(3):
        nc.vector.bn_stats(stats[:, sg], x_ap[:, sg * 256:(sg + 1) * 256])
    mv = pool.tile([128, 2], F32, tag="ln_mv")
    nc.vector.bn_aggr(mv, stats)
    lnv = pool.tile([128, 1], F32, tag="ln_lnv")
    nc.scalar.activation(lnv, mv[:, 1:2], Act.Ln, bias=eps_col)
    rstd = pool.tile([128, 1], F32, tag="ln_rstd")
    nc.scalar.activation(rstd, lnv, Act.Exp, scale=-0.5)
    nmr = pool.tile([128, 1], F32, tag="ln_nmr")
    nc.vector.tensor_scalar(out=nmr, in0=mv[:, 0:1], scalar1=rstd,
                            scalar2=-1.0, op0=mybir.AluOpType.mult,
                            op1=mybir.AluOpType.mult)
    nc.scalar.activation(out_ap, x_ap, Act.Identity, scale=rstd, bias=nmr)


def _build(has_f2b, w1_scale, w2_scale, reps=1):
    nc = bacc.Bacc("TRN2", target_bir_lowering=False, debug=False,
                   num_devices=NCORES)
    x_d = nc.dram_tensor("x", [NTOK, DIM], F32, kind="ExternalInput")
    xb_d = nc.dram_tensor("xb", [NTOK, DIM], BF16, kind="ExternalInput")
    wqkv_d = nc.dram_tensor("wqkv", [128, 6, 3 * DIM], BF16, kind="ExternalInput")
    qkb_d = nc.dram_tensor("qkb", [128, 12], F32, kind="ExternalInput")
    wproj_d = nc.dram_tensor("wproj", [128, 6, DIM], BF16, kind="ExternalInput")
    w1_d = nc.dram_tensor("w1", [128, 6, HID], FP8, kind="ExternalInput")
    b1_d = nc.dram_tensor("b1c", [128, 24], F32, kind="ExternalInput")
    w2_d = nc.dram_tensor("w2", [128, 24, DIM], FP8, kind="ExternalInput")
    if has_f2b:
        f2b_d = nc.dram_tensor("f2b", [128, DIM], F32, kind="ExternalInput")
    qext_d = nc.dram_tensor("qext", [KEXT, HEADS, N], BF16,
                            kind="ExternalInput")
    kext_d = nc.dram_tensor("kext", [KEXT, HEADS, N], BF16,
                            kind="ExternalInput")
    y_d = nc.dram_tensor("y", [NTOK, DIM], F32, kind="ExternalOutput")

    Act = mybir.ActivationFunctionType
    Alu = mybir.AluOpType

    with tile.TileContext(nc) as tc:
        with (
            tc.tile_pool(name="consts", bufs=1) as cp,
            tc.tile_pool(name="wts", bufs=1) as wp,
            tc.tile_pool(name="small", bufs=4) as sp,
            tc.tile_pool(name="xio", bufs=3) as xp,
            tc.tile_pool(name="big", bufs=1) as bp,
            tc.tile_pool(name="perimg", bufs=1) as ip,
            tc.tile_pool(name="gelu", bufs=1) as gp,
            tc.tile_pool(name="ptile", bufs=3) as pp,
            tc.tile_pool(name="dram", bufs=1, space="DRAM") as dp,
            tc.tile_pool(name="psA", bufs=2, space="PSUM") as psA,
            tc.tile_pool(name="psT", bufs=2, space="PSUM") as psT,
            tc.tile_pool(name="psS", bufs=2, space="PSUM") as psS,
        ):
            # ---- constants -------------------------------------------------
            # Pre-load the exp+ln act table (set 6: natural_log_exp_and_
            # others) so the table-load pass sees every Ln/Exp/Identity
            # covered and doesn't thrash between the ln-only and exp-only
            # sets (measured 146 ACT_TABLE_LOADs = 187us without this).
            nc.scalar.add_instruction(mybir.InstLoadActFuncSet(
                name=nc.get_next_instruction_name(), ins=[], outs=[],
                act_func_set_id=6))
            ident = cp.tile([128, 128], BF16)
            make_identity(nc, ident)
            eps_col = cp.tile([128, 1], F32)
            nc.vector.memset(eps_col, EPS)
            qkb_sb = cp.tile([128, 12], F32)
            nc.scalar.dma_start(qkb_sb, qkb_d[:])
            b1_sb = cp.tile([128, 24], F32)
            nc.scalar.dma_start(b1_sb, b1_d[:])
            if has_f2b:
                f2b_sb = cp.tile([128, DIM], F32)
                nc.scalar.dma_start(f2b_sb, f2b_d[:])
            def _body():
                    # wproj/w2 are persistent (own SBUF, DMA'd up front on the
                    # idle gpsimd queue); wqkv -> w1 share one rotating slot.
                    wproj_sb = cp.tile([128, 6, DIM], BF16, name="wproj_sb")
                    # single-shot weight DMAs: each dma_start occupies the
                    # issuing engine's sequencer for ~1-4us generating
                    # descriptors, so chunked loads serialize behind real
                    # compute. One descriptor per weight is optimal (the
                    # host layouts are contiguous per partition).
                    wqkv_sb = wp.tile([128, 6, 3 * DIM], BF16, tag="wslotA")
                    nc.scalar.dma_start(wqkv_sb, wqkv_d[:])
                    w1_sb = wp.tile([128, 6, HID], FP8, tag="wslotB")

                    # ---- persistent activations -----------------------------------
                    hT = bp.tile([128, 6, NTOKP], BF16, tag="featmaj")
                    attn_oT = bp.tile([128, 6, NTOKP], BF16)
                    h2T = bp.tile([128, 6, NTOKP], FP8)
                    x1_dram = dp.tile([NTOKP, DIM], F32)

                    # ---- phase A: LN1 + transpose to hT (issued lazily, ----------
                    # interleaved with the attention pipeline below so the LN
                    # chain overlaps PE work instead of stalling it)
                    def _phaseA_tile(t):
                        rows = min(128, NTOK - t * 128)
                        x_t = xp.tile([128, DIM], BF16, tag="xb_t",
                                      name="xA_t")
                        if rows < 128:
                            nc.vector.memset(x_t, 0.0)
                        nc.sync.dma_start(x_t[0:rows], xb_d[t * 128:t * 128 + rows])
                        stats = sp.tile([128, 3, 6], F32, tag="ln_stats")
                        for sg in range(3):
                            nc.vector.bn_stats(stats[:, sg],
                                               x_t[:, sg * 256:(sg + 1) * 256])
                        mv = sp.tile([128, 2], F32, tag="ln_mv")
                        nc.vector.bn_aggr(mv, stats)
                        lnv = sp.tile([128, 1], F32, tag="ln_lnv")
                        nc.scalar.activation(lnv, mv[:, 1:2], Act.Ln, bias=eps_col)
                        rstd = sp.tile([128, 1], F32, tag="ln_rstd")
                        nc.scalar.activation(rstd, lnv, Act.Exp, scale=-0.5)
                        h_t = xp.tile([128, DIM], BF16, tag="h_t", name="hA_t",
                                      bufs=2)
                        nc.vector.tensor_scalar(out=h_t, in0=x_t,
                                                scalar1=mv[:, 0:1], scalar2=rstd,
                                                op0=Alu.subtract, op1=Alu.mult)
                        for c in range(6):
                            ps_t = psT.tile([128, 128], BF16, tag="tp",
                                            name="psA_t")
                            nc.tensor.transpose(ps_t, h_t[:, c * 128:(c + 1) * 128],
                                                ident)
                            if c < 3:
                                nc.scalar.copy(hT[:, c, t * 128:(t + 1) * 128],
                                               ps_t)
                            else:
                                nc.vector.tensor_copy(
                                    hT[:, c, t * 128:(t + 1) * 128], ps_t)

                    A_state = [0]

                    def _ensure_A(upto):
                        while A_state[0] <= min(upto, NTILES - 1):
                            _phaseA_tile(A_state[0])
                            A_state[0] += 1

                    # ---- phase B+C: per image QKV + attention ---------------------
                    # get the first phase-A tiles going before anything else
                    # queues on Act/DVE (startup latency).
                    _ensure_A(3)   # tiles for images 0-1
                    nc.vector.memset(attn_oT[:, :, NTOK:NTOKP], 0.0)
                    # persistent ping-pong buffers: the rel-pos extension rows
                    # and the ones block (softmax denominator trick) are
                    # constant across images -- load/memset them once. The
                    # ext DMAs go on the otherwise idle gpsimd queue so they
                    # never block phase A's Act ops.
                    qT_b = [ip.tile([128, HEADS, N], BF16, tag=f"qT{j}",
                                   name=f"qT{j}") for j in range(2)]
                    kT_b = [ip.tile([128, HEADS, N], BF16, tag=f"kT{j}",
                                    name=f"kT{j}") for j in range(2)]
                    v_b = [ip.tile([128, 2, HEADS, 128], BF16, tag=f"v{j}",
                                   name=f"v{j}") for j in range(2)]
                    for j in range(2):
                        nc.gpsimd.dma_start(qT_b[j][64:64 + KEXT, :, :], qext_d[:])
                        nc.gpsimd.dma_start(kT_b[j][64:64 + KEXT, :, :], kext_d[:])
                        nc.vector.memset(v_b[j][:, :, :, 64:128], 1.0)

                    def _qk_mm(i, m, act_evac):
                        qcols_i = slice(i * N, (i + 1) * N)
                        ps = psA.tile([128, 512], F32, tag="mm512",
                                      name="ps_qk")
                        for c in range(6):
                            nc.tensor.matmul(ps[:, 0:N],
                                             wqkv_sb[:, c, m * 128:(m + 1) * 128],
                                             hT[:, c, qcols_i],
                                             start=(c == 0), stop=(c == 5))
                        dst = qT_b[i % 2] if m < 6 else kT_b[i % 2]
                        hh = 2 * (m % 6)
                        if act_evac:
                            nc.scalar.activation(
                                dst[0:64, hh, :], ps[0:64, 0:N], Act.Identity,
                                bias=qkb_sb[0:64, m:m + 1])
                        else:
                            nc.vector.tensor_scalar_add(
                                out=dst[0:64, hh, :], in0=ps[0:64, 0:N],
                                scalar1=qkb_sb[0:64, m:m + 1])
                        nc.vector.tensor_scalar_add(
                            out=dst[0:64, hh + 1, :], in0=ps[64:128, 0:N],
                            scalar1=qkb_sb[64:128, m:m + 1])

                    def _v_mm(i):
                        v_i = v_b[i % 2]
                        for st in range(2):
                            tok0 = i * N + st * 128
                            ksz = min(128, (i + 1) * N - tok0)
                            for ns, w in enumerate([512, 256]):
                                ps = psA.tile([128, 512], F32, tag="mm512",
                                              name="ps_v")
                                for c in range(6):
                                    nc.tensor.matmul(
                                        ps[0:ksz, 0:w],
                                        hT[:, c, tok0:tok0 + ksz],
                                        wqkv_sb[:, c, 2 * DIM + ns * 512:
                                                2 * DIM + ns * 512 + w],
                                        start=(c == 0), stop=(c == 5))
                                nh = w // 64
                                nc.vector.tensor_copy(
                                    v_i[0:ksz, st, ns * 8:ns * 8 + nh, 0:64],
                                    ps[0:ksz, 0:w].rearrange(
                                        "k (h d) -> k h d", d=64))

                    def _scores_exp(i, h):
                        qT_i, kT_i = qT_b[i % 2], kT_b[i % 2]
                        ps_s = psS.tile([128, 4, 256], F32, tag="att",
                                        name="ps_s")
                        for j, hh in enumerate((h, h + 6)):
                            for st in range(2):
                                tok0 = i * N + st * 128
                                ksz = min(128, (i + 1) * N - tok0)
                                lt = tok0 - i * N
                                nc.tensor.matmul(ps_s[0:ksz, 2 * j + st, 0:N],
                                                 kT_i[0:64 + KEXT, hh, lt:lt + ksz],
                                                 qT_i[0:64 + KEXT, hh, :],
                                                 start=True, stop=True)
                        p_t = pp.tile([128, 4, N], BF16, tag="p_t", name="p_t",
                                      bufs=2)
                        nc.scalar.activation(p_t, ps_s[:, :, 0:N], Act.Exp)
                        return p_t

                    def _av_norm(i, h, p_t):
                        qcols_i = slice(i * N, (i + 1) * N)
                        v_i = v_b[i % 2]
                        av = psA.tile([128, 512], F32, tag="mm512",
                                      name="ps_av")
                        ps_av = av[:, 0:2 * N].rearrange("p (j n) -> p j n", n=N)
                        for j, hh in enumerate((h, h + 6)):
                            for st in range(2):
                                tok0 = i * N + st * 128
                                ksz = min(128, (i + 1) * N - tok0)
                                nc.tensor.matmul(ps_av[:, j],
                                                 v_i[0:ksz, st, hh, :],
                                                 p_t[0:ksz, 2 * j + st],
                                                 start=(st == 0), stop=(st == 1))
                        # 1/den = exp(-ln(den)) on Act: ~0.9us vs 3.1us for
                        # the iterative DVE reciprocal; exp/ln table stays
                        # loaded (see the InstLoadActFuncSet above).
                        lnden = sp.tile([128, 2, N], F32, tag="lnden", bufs=1)
                        nc.scalar.activation(lnden[0:64], ps_av[64:128], Act.Ln)
                        den = pp.tile([128, 2, N], F32, tag="den", name="den",
                                      bufs=2)
                        nc.scalar.activation(den[0:64], lnden[0:64], Act.Exp,
                                             scale=-1.0)
                        ao = attn_oT[(h % 2) * 64:(h % 2) * 64 + 64, h // 2,
                                     qcols_i]
                        out2 = bass.AP(tensor=ao.tensor, offset=ao.offset,
                                       ap=[list(ao.ap[0]), [3 * NTOKP, 2], [1, N]])
                        nc.vector.tensor_tensor(out=out2, in0=ps_av[0:64],
                                                in1=den[0:64], op=Alu.mult)

                    # ---- phase D/E (emitted inside the pipeline below) -----------
                    xD_pend = {}
                    xF_pend = {}

                    def _prefetch_xD(t):
                        if t >= NTILES:
                            return
                        rows = min(128, NTOK - t * 128)
                        x_t = xp.tile([128, DIM], F32, tag="x_t", name="xD_t")
                        if rows < 128:
                            nc.vector.memset(x_t, 0.0)
                        # x_d already carries x + proj bias (host-folded)
                        nc.sync.dma_start(x_t[0:rows], x_d[t * 128:t * 128 + rows])
                        xD_pend[t] = x_t

                    def _prefetch_xF(t):
                        x_t = xp.tile([128, DIM], F32, tag="xF", name="xF_t",
                                      bufs=2)
                        nc.sync.dma_start(x_t, x1_dram[t * 128:(t + 1) * 128])
                        xF_pend[t] = x_t

                    def _de_tile(t):
                        x_t = xD_pend.pop(t)
                        x1_t = xp.tile([128, DIM], F32, tag="y_sb", name="x1_t")
                        for ns, w in enumerate([512, 256]):
                            sl = slice(ns * 512, ns * 512 + w)
                            ps = psA.tile([128, 512], F32, tag="mm512",
                                          name="ps_pr")
                            for c in range(6):
                                nc.tensor.matmul(ps[:, 0:w],
                                                 attn_oT[:, c, t * 128:(t + 1) * 128],
                                                 wproj_sb[:, c, sl],
                                                 start=(c == 0), stop=(c == 5))
                            nc.vector.tensor_tensor(out=x1_t[:, sl], in0=ps[:, 0:w],
                                                    in1=x_t[:, sl], op=Alu.add)
                        nc.sync.dma_start(x1_dram[t * 128:(t + 1) * 128], x1_t)
                        _prefetch_xD(t + 2)
                        h_t = xp.tile([128, DIM], BF16, tag="h_t", name="hD_t",
                                      bufs=2)
                        _ln_apply(nc, sp, x1_t, h_t, eps_col)
                        for c in range(6):
                            ps_t = psT.tile([128, 128], BF16, tag="tp",
                                            name="psD_t")
                            nc.tensor.transpose(ps_t, h_t[:, c * 128:(c + 1) * 128],
                                                ident)
                            if c < 2:
                                nc.vector.tensor_copy(
                                    h2T[:, c, t * 128:(t + 1) * 128], ps_t)
                            else:
                                nc.scalar.copy(
                                    h2T[:, c, t * 128:(t + 1) * 128], ps_t)

                    de_state = [0]

                    def _emit_de(i):
                        # proj/LN2 for every token tile fully covered by the
                        # images whose attention output is already written --
                        # this is the PE filler that keeps HAM warm during
                        # attention and removes the standalone D/E phase.
                        while (de_state[0] < NTILES and
                               min((de_state[0] + 1) * 128, NTOK) <= 197 * i):
                            _de_tile(de_state[0])
                            de_state[0] += 1

                    # software pipeline, 3 stages deep: image i's QKV matmuls
                    # fill the PE while image i-1's exps drain on Act, AV of
                    # head-pair h-1 runs one step behind its exp, and proj/LN2
                    # tiles of finished images trail the attention.
                    def _tmax(j):
                        return (197 * (j + 1) - 1) // 128

                    pt_h = {}
                    for i in range(NB + 1):
                        for h in range(6):
                            if i > 0:
                                pt_h[h] = _scores_exp(i - 1, h)
                            if i < NB:
                                _qk_mm(i, 2 * h, act_evac=True)
                                _qk_mm(i, 2 * h + 1, act_evac=False)
                            if i > 0 and h > 0:
                                _av_norm(i - 1, h - 1, pt_h[h - 1])
                        if i > 0:
                            _av_norm(i - 1, 5, pt_h[5])
                        if i == 5:
                            _prefetch_xD(0)
                            _prefetch_xD(1)
                        if i < NB:
                            _v_mm(i)
                            _ensure_A(_tmax(min(i + 2, NB - 1)))
                        if i == 1:
                            # gate w1/wproj DMAs behind image 0's v tile (WAW
                            # deps through the dst tiles themselves, so the
                            # scheduler can't hoist the DMAs into the startup
                            # HBM crunch; both are needed only at phase D/F)
                            nc.gpsimd.tensor_copy(w1_sb[0:1, 0, 0:1],
                                                  v_b[0][0:1, 0, 0, 0:1])
                            nc.gpsimd.dma_start(w1_sb, w1_d[:])
                            nc.gpsimd.tensor_copy(wproj_sb[0:1, 0, 0:1],
                                                  v_b[0][0:1, 0, 0, 0:1])
                            nc.gpsimd.dma_start(wproj_sb, wproj_d[:])

                    # D/E as its own block (interleaving it into the attention
                    # pipeline overloaded Act/DVE, measured slower)
                    _emit_de(NB)

                    # ---- phase F: MLP --------------------------------------------
                    # w2 reuses wqkv's wbig slot (free once attention ends)
                    w2_sb = wp.tile([128, 24, DIM], FP8, tag="wslotA")
                    nc.gpsimd.dma_start(w2_sb, w2_d[:])

                    def _fc1_chunk(g, gT, mcs):
                        w = NSL[g]
                        col0 = g * 512
                        for mc in mcs:
                            ps = psA.tile([128, 512], F32, tag="mm512",
                                          name="ps_f1")
                            # fp8 DoubleRow: each matmul contracts 2 k-tiles
                            # (256 deep); w1 pre-scaled by w1_scale on host,
                            # gelu's scale undoes it.
                            for c3 in range(3):
                                nc.tensor.matmul(
                                    ps[:, 0:w],
                                    w1_sb[:, 2 * c3:2 * c3 + 2,
                                          mc * 128:(mc + 1) * 128],
                                    h2T[:, 2 * c3:2 * c3 + 2, col0:col0 + w],
                                    start=(c3 == 0), stop=(c3 == 2),
                                    perf_mode=mybir.MatmulPerfMode.DoubleRow)
                            nc.scalar.activation(gT[:, mc, 0:w], ps[:, 0:w],
                                                 Act.Gelu,
                                                 bias=b1_sb[:, mc:mc + 1],
                                                 scale=1.0 / w1_scale)

                    def _fc2_tile(g, gT, tt):
                        t = g * 4 + tt
                        rows = min(128, NTOK - t * 128)
                        x_t = xF_pend.pop(t)
                        y_sb = xp.tile([128, DIM], F32, tag="y_sb", name="yF_t")
                        for ns2, w2 in enumerate([512, 256]):
                            sl = slice(ns2 * 512, ns2 * 512 + w2)
                            ps = psA.tile([128, 512], F32, tag="mm512",
                                          name="ps_f2")
                            # fp8 DoubleRow over 12 k-tile pairs; w2 is
                            # host-scaled by w2_scale, descale on Act, then
                            # the residual add on DVE.
                            for k2 in range(12):
                                nc.tensor.matmul(
                                    ps[:, 0:w2],
                                    gT[:, 2 * k2:2 * k2 + 2,
                                       tt * 128:(tt + 1) * 128],
                                    w2_sb[:, 2 * k2:2 * k2 + 2, sl],
                                    start=(k2 == 0), stop=(k2 == 11),
                                    perf_mode=mybir.MatmulPerfMode.DoubleRow)
                            nc.scalar.activation(y_sb[:, sl], ps[:, 0:w2],
                                                 Act.Identity,
                                                 scale=1.0 / w2_scale)
                            nc.vector.tensor_tensor(out=y_sb[:, sl],
                                                    in0=y_sb[:, sl],
                                                    in1=x_t[:, sl], op=Alu.add)
                        if has_f2b:
                            nc.vector.tensor_tensor(out=y_sb, in0=y_sb, in1=f2b_sb,
                                                    op=Alu.add)
                        nc.sync.dma_start(y_d[t * 128:t * 128 + rows], y_sb[0:rows])

                    for g in range(4):
                        ntg = NSL[g] // 128
                        gT_cur = gp.tile([128, 24, 512], FP8, tag="gT",
                                         name="gT_g")
                        for k in range(4):
                            _fc1_chunk(g, gT_cur, range(6 * k, 6 * k + 6))
                            if k < ntg:
                                _prefetch_xF(g * 4 + k)
                        for tt in range(ntg):
                            _fc2_tile(g, gT_cur, tt)

            if reps == 1:
                _body()
            else:
                import os
                _unroll = int(os.environ.get("BENCH_UNROLL", "1"))
                _stag = os.environ.get("BENCH_STAG", "0") == "1"
                with tc.For_i(0, reps // _unroll, 1,
                              staggered_reset=_stag):
                    for _u in range(_unroll):
                        _body()

    nc.compile()
    return nc


def kernel(**inputs) -> np.ndarray:
    x = np.asarray(inputs["x"], np.float32)          # [64, 197, 768]
    consts = _host_prep(inputs)
    key = ("blk", consts["has_f2b"], consts["w1_scale"], consts["w2_scale"])
    if key not in _nc_cache:
        _nc_cache[key] = _build(consts["has_f2b"], consts["w1_scale"],
                                consts["w2_scale"])
    nc = _nc_cache[key]

    shared = {k: consts[k] for k in ("wqkv", "qkb", "wproj", "w1",
                                     "b1c", "w2", "f2b", "qext", "kext")}
    if not consts["has_f2b"]:
        shared.pop("f2b")
    pb_row = consts["pb_row"]
    in_maps = []
    for c in range(NCORES):
        m = dict(shared)
        xc = np.ascontiguousarray(
            x[c * NB:(c + 1) * NB].reshape(NTOK, DIM))
        m["x"] = xc + pb_row[None, :]   # proj bias folded into residual
        m["xb"] = xc.astype(NPBF16)
        in_maps.append(m)

    res = run_bass_kernel_spmd(nc, in_maps, core_ids=list(range(NCORES)))
    out = np.empty((B, N, DIM), np.float32)
    for c in range(NCORES):
        out[c * NB:(c + 1) * NB] = res.results[c]["y"].reshape(NB, N, DIM)
    return out
